# revision 32
# baseline (speedup 1.0000x reference)
"""Full-width attention (B=4, S=2048, D=1024, no head split) on 8 TRN2 cores.

Sharding: data-parallel over (batch, query-half) -> 8 shards. Core c handles
batch b = c//2, query rows [h*1024, (h+1)*1024) with h = c%2.

Algorithm (FLOP-minimal per core: 12.9 GF = 768 N=512 matmuls, bf16):
  - Fold Wq/Wk into the QUERY side: Q'' = (x_q M + w3) / 8 with M = Wq^T Wk
    and w3 = Wk^T bq. scores^T[k,q] = sum_e x[k,e] Q''[q,e] needs no key
    projection. Softmax without max-subtraction (|scores| <= ~25).
  - out[q,e] = (x^T E)^T Wv^T scaled by 1/rowsum + bv.
  - All big matmuls bf16 x bf16 (216 ns/MM at N=512, warm): PE streams
    gap-free at ~99% occupancy for the whole compute window.

Round-2/3/4 changes vs the 187.0us version (trace-driven; now ~182.7-183.4
in the 2.4GHz chip state):
  - Warmup = 9 MMs, tuned so the ones-warmup ends exactly at the first
    m-chunk arrival (~11.9us, HBM-contention-pinned across the 8 cores)
    AND pre-burns the full ~3.4us HAM window, so ko0 runs warm at 216
    ns/MM instead of cold at 427. After that the 768-MM stream is
    gap-free to the end (verified <0.5us total stalls).
  - xt loads split into qc-halves and the xt-b half gated behind an
    m7 blocker read: the scalar ring's critical piece is 1KB/ko (vs m's
    2KB) and the ring goes idle after xt-a, so the SDMA round-robin
    gives the m ring the full HBM share for m2-m7 (~1us/ko faster
    arrival; lastMM moved ~1us earlier, reproducibly). A finer JIT
    throttle (blocker per xt-a chunk) over-serialized the scalar queue
    and delayed the qc0 evacuations — net loss, don't revisit.
  - The xt-tail/wv/xn DMA issues moved off the Scalar queue: each issue
    costs 600-800ns of queue-engine time and was serializing with the
    qc0 PSUM evacuations (ACTIVATE), stalling qc1's first MMs ~1us.
  - qc0/qc1 evacuations alternate Scalar (activation+bias) and DVE
    (tensor_scalar_add): evac rate 432 -> ~216 ns/bank matches the MM
    consume rate at the sweep boundary.
  - Phase 4 last group de-interleaved (b fully, then c split in 2x256)
    so the post-last-MM tail is one small fuse + 64KB DMA instead of
    two serialized 512-col fuses + 128KB DMAs.
  - Round 4: xt-b as 2 coarse dmas and groups g0-g5's output as one
    256KB dma each (both have multi-us slack): fewer dmas reduce queue
    and teardown bookkeeping; measured best dropped 183.3 -> 182.7.
    Re-tried m0 half-split WITH the blocker (start ~11.8 vs 11.9): the
    extra dma's completion overhead stalls ko2-3 ~1.4us on bad runs —
    net negative (183.6 mean vs 183.1), reverted.
  - Round 5: the final 64KB out piece ships as two 32KB dmas on BOTH
    rings (sync + scalar issues run concurrently; scalar ring idle from
    phase 3 on) — parallelizes the ~600ns issue and the transfer in the
    exposed tail. Config locked at 182.7-183.4us.
Measured invariants (don't re-litigate): per-NC HBM ~320-360GB/s total,
~160/ring steady, first-chunk completion ~11.9us (ring-arm + contention);
a 3rd DMA path (gpsimd SWDGE) causes 3-way contention, stalls ko1-3 by
~5us and re-throttles HAM -- strictly worse. Phase-1's end is pinned by
TOTAL 4MB arrival (~25.8us), so no phase-1 restructure (finer chunks,
eo/qc generations) can beat the current ko-outer sweep. fp8/DoubleRow is
precision-infeasible everywhere (needs <=2e-2 l2; e4m3 alone adds ~3.6%).
Periodic +163ns PE hiccups every 10.79us (~2.3us total) are firmware.
Run-to-run: chip sometimes drops to 2.0GHz (P0 power state) -> all MMs
259ns and ~+35us total; not code-dependent, ignore those samples.
"""

import math
from contextlib import ExitStack

import numpy as np

P = 128
B, S, D = 4, 2048, 1024
SQ = 1024  # query rows per core
KO8 = 8  # 1024 contraction / 128
KO16 = 16  # 2048 contraction / 128
N_CORES = 8
N_WARM = 9


def build_bass():
    from concourse import bacc
    import concourse.mybir as mybir
    from concourse.tile import TileContext

    f32 = mybir.dt.float32
    f32r = mybir.dt.float32r
    bf16 = mybir.dt.bfloat16
    AF = mybir.ActivationFunctionType

    nc = bacc.Bacc(
        "TRN2",
        target_bir_lowering=False,
        debug=False,
        enable_asserts=False,
        num_devices=N_CORES,
    )

    xT = nc.dram_tensor("xT", [D, S], bf16, kind="ExternalInput")
    xn = nc.dram_tensor("xn", [S, D], bf16, kind="ExternalInput")
    mT = nc.dram_tensor("mT", [D, D], bf16, kind="ExternalInput")
    wvT = nc.dram_tensor("wvT", [D, D], bf16, kind="ExternalInput")
    w3 = nc.dram_tensor("w3", [P, KO8], f32, kind="ExternalInput")
    bvb = nc.dram_tensor("bvb", [P, D], f32, kind="ExternalInput")
    # bf16 output halves the out-DMA traffic (the tail's exposed transfer);
    # host converts back to f32 — quantization ~0.2%, well inside margin
    out = nc.dram_tensor("out", [SQ, D], bf16, kind="ExternalOutput")

    xT_r = xT[:, :].rearrange("(ko p) s -> p ko s", p=P)
    xn_r = xn[:, :].rearrange("(ko p) d -> p ko d", p=P)
    mT_r = mT[:, :].rearrange("(ko p) e -> p ko e", p=P)
    wvT_r = wvT[:, :].rearrange("(ko p) e -> p ko e", p=P)

    with TileContext(nc) as tc, ExitStack() as ctx:
        cst_p = ctx.enter_context(tc.tile_pool(name="cst", bufs=1))
        big_p = ctx.enter_context(tc.tile_pool(name="big", bufs=1))
        out_p = ctx.enter_context(tc.tile_pool(name="osp", bufs=3))
        psA_p = ctx.enter_context(tc.tile_pool(name="psA", bufs=3, space="PSUM"))
        psB_p = ctx.enter_context(tc.tile_pool(name="psB", bufs=2, space="PSUM"))
        psC_p = ctx.enter_context(tc.tile_pool(name="psC", bufs=2, space="PSUM"))
        psR_p = ctx.enter_context(tc.tile_pool(name="psR", bufs=1, space="PSUM"))
        dram_p = ctx.enter_context(tc.tile_pool(name="drp", bufs=1, space="DRAM"))

        # warmup operand comes from a memset, not a DMA (walrus rejects
        # memset on f32r tiles, so memset f32 and bitcast for the PE)
        ones_f = cst_p.tile([P, 512], f32, tag="ones", name="ones_f")
        nc.vector.memset(ones_f[:], 1.0)
        ones_t = ones_f[:, :].bitcast(f32r)
        w3_t = cst_p.tile([P, KO8], f32, tag="w3", name="w3_t")
        bvb_t = cst_p.tile([P, D], f32, tag="bvb", name="bvb_t")

        # big residents
        xt_sb = big_p.tile([P, KO8, S], bf16, tag="xt", name="xt_sb")
        xn_sb = big_p.tile([P, KO16, D], bf16, tag="xn", name="xn_sb")
        m_sb = big_p.tile([P, KO8, D], bf16, tag="m", name="m_sb")
        wv_sb = big_p.tile([P, KO8, D], bf16, tag="wv", name="wv_sb")
        qt_sb = big_p.tile([P, KO8, SQ], bf16, tag="qt", name="qt_sb")
        e_sb = [
            big_p.tile([P, KO16, 512], bf16, tag=f"E{qc}", name=f"e_sb{qc}")
            for qc in range(2)
        ]
        px_sb = big_p.tile([P, KO8, SQ], bf16, tag="px", name="px_sb")
        racc = [
            cst_p.tile([P, 512], f32r, tag=f"racc{qc}", name=f"racc{qc}")
            for qc in range(2)
        ]
        rs_dram = dram_p.tile([1, SQ], f32, tag="rsd", name="rs_dram")

        # Phase-1 feed: per-ko full-width chunks (one dma each — finer splits
        # pay ~0.4us/dma completion overhead), m on the sync ring and x^T
        # query columns on the scalar ring in parallel. All non-phase-1
        # loads ride the SYNC ring behind the m chunks: their ~700ns issue
        # slots must not serialize with the qc0 evacuations on the Scalar
        # engine queue, and their transfers stay behind the critical chunks
        # in ring-FIFO order. (SWDGE as a 3rd path is a net loss — per-NC
        # HBM caps ~320-360 GB/s total.)
        # xt comes as qc-halves: the qc0 sweep needs only cols 0:512 per ko,
        # so the scalar ring's per-ko critical piece is 1KB/partition vs the
        # m ring's 2KB — the m ring alone gates the sweep, removing the
        # max-of-two-rings arrival jitter. The xt-b tail (+8 dmas of
        # overhead) is only needed by the qc1 sweep, which has ~12us slack.
        for ko in range(KO8):
            nc.sync.dma_start(m_sb[:, ko, :], mT_r[:, ko, :])
            nc.scalar.dma_start(xt_sb[:, ko, 0:512], xT_r[:, ko, 0:512])
        # small consts on the scalar ring (w3 first used ~22us in)
        nc.scalar.dma_start(w3_t[:], w3[:, :])
        # blocker: hold the xt-b issues until the last m chunk has landed.
        # SDMA engines round-robin only among queues WITH work, so an idle
        # scalar ring gives the m ring the full HBM share during the qc0
        # sweep's critical window (m paces 1.6-1.7us/ko vs 1.73 consumed).
        # xt-b is first needed by the qc1 sweep (~12us of slack).
        blk_t = cst_p.tile([1, 1], f32, tag="blk", name="blk_t")
        nc.scalar.copy(blk_t[:], m_sb[0:1, KO8 - 1, D - 1 : D])
        # xt-b as 2 coarse dmas: it has >5us of slack before the qc1 sweep
        # consumes it, and fewer dmas mean less issue time ahead of the
        # qc0 evacuations on this queue (and a hair less teardown work)
        for kp in range(2):
            nc.scalar.dma_start(
                xt_sb[:, 4 * kp : 4 * kp + 4, 512:SQ],
                xT_r[:, 4 * kp : 4 * kp + 4, 512:SQ],
            )
        nc.scalar.dma_start(bvb_t[:], bvb[:, :])
        # non-critical bulk loads: sync ring, behind the m halves
        for kp in range(2):
            nc.sync.dma_start(
                xt_sb[:, 4 * kp : 4 * kp + 4, SQ:S],
                xT_r[:, 4 * kp : 4 * kp + 4, SQ:S],
            )
        nc.sync.dma_start(wv_sb[:, :, :], wvT_r[:, :, :])
        for kp in range(2):
            nc.sync.dma_start(
                xn_sb[:, 8 * kp : 8 * kp + 8, :], xn_r[:, 8 * kp : 8 * kp + 8, :]
            )

        # PE warm-up on the ones tile: 9 MMs end right at the first m-chunk
        # arrival (~11.9us) and pre-burn the full ~3.4us HAM window with
        # ~0.6us margin, so the real stream starts warm (216ns/MM).
        warm = psR_p.tile([1, 512], f32, tag="psR", name="warm")
        for _ in range(N_WARM):
            nc.tensor.matmul(warm[:], ones_t[:, 0:1], ones_t[:, :])

        # ---- Phase 1: Q''T[e, q] = M^T x_q^T + w3 (scaled by 1/8 on host) --
        # ko-OUTER with all 8 eo accumulations held open across the full
        # PSUM bank set: each ko step needs only chunk ko of m/x^T, so the
        # whole sweep paces with the DMA chunk arrivals. (4-pass eo/qc-half
        # restructures with split/merged DMAs were measured at 184.9-187.9
        # vs 184.0-186.3 for this layout: per-dma completion overhead
        # (~0.4us) and cross-ring HBM contention eat the theoretical gain.)
        def q_sweep(qc):
            banks = [
                psA_p.tile([P, 512], f32, tag="psA", name=f"qp{qc}a{i}")
                for i in range(3)
            ] + [
                psB_p.tile([P, 512], f32, tag="psB", name=f"qp{qc}b{i}")
                for i in range(2)
            ] + [
                psC_p.tile([P, 512], f32, tag="psC", name=f"qp{qc}c{i}")
                for i in range(2)
            ] + [psR_p.tile([P, 512], f32, tag="psR", name=f"qp{qc}r")]
            for ko in range(KO8):
                for eo in range(KO8):
                    nc.tensor.matmul(
                        banks[eo][:],
                        m_sb[:, ko, eo * P : (eo + 1) * P],
                        xt_sb[:, ko, qc * 512 : (qc + 1) * 512],
                        start=(ko == 0), stop=(ko == KO8 - 1),
                    )
            # evacuations alternate Scalar/DVE so the next sweep's first MMs
            # (which reuse these banks) aren't rate-limited by one engine
            for eo in range(KO8):
                dst = qt_sb[:, eo, qc * 512 : (qc + 1) * 512]
                if eo % 2 == 0:
                    nc.scalar.activation(
                        dst, banks[eo][:], AF.Identity, bias=w3_t[:, eo : eo + 1]
                    )
                else:
                    nc.vector.tensor_scalar_add(
                        dst, banks[eo][:], w3_t[:, eo : eo + 1]
                    )

        q_sweep(0)
        q_sweep(1)

        # ---- Phase 2: scores^T -> exp -> E (bf16), rowsum acc on DVE ------
        for kidx in range(KO16):
            pa = psA_p.tile([P, 512], f32, tag="psA", name="spa")
            pb = psB_p.tile([P, 512], f32, tag="psB", name="spb")
            for eo in range(KO8):
                lh = xt_sb[:, eo, kidx * P : (kidx + 1) * P]
                nc.tensor.matmul(
                    pa[:], lh, qt_sb[:, eo, 0:512],
                    start=(eo == 0), stop=(eo == KO8 - 1),
                )
                nc.tensor.matmul(
                    pb[:], lh, qt_sb[:, eo, 512:1024],
                    start=(eo == 0), stop=(eo == KO8 - 1),
                )
            nc.scalar.activation(e_sb[0][:, kidx, :], pa[:], AF.Exp)
            nc.scalar.activation(e_sb[1][:, kidx, :], pb[:], AF.Exp)
            for qc in range(2):
                if kidx == 0:
                    nc.vector.tensor_copy(racc[qc][:], e_sb[qc][:, 0, :])
                else:
                    nc.vector.tensor_add(
                        racc[qc][:], racc[qc][:], e_sb[qc][:, kidx, :]
                    )

        # ---- Phase 3: PX^T[d, q] = sum_k x[k, d] E[k, q] -------------------
        for dc in range(KO8):
            pp = psA_p.tile([P, 512], f32, tag="psA", name="ppx")
            for ko in range(KO16):
                nc.tensor.matmul(
                    pp[:],
                    xn_sb[:, ko, dc * P : (dc + 1) * P],
                    e_sb[0][:, ko, :],
                    start=(ko == 0), stop=(ko == KO16 - 1),
                )
            nc.scalar.copy(px_sb[:, dc, 0:512], pp[:])

        # rowsum partition-reduce + [1,1024] -> [128,8] recip via DRAM bounce
        # (PE cost ~2 tiny matmuls; bounce hides under PX)
        for qc in range(2):
            pr = psR_p.tile([1, 512], f32, tag="psR", name="pr")
            nc.tensor.matmul(pr[:], ones_t[:, 0:1], racc[qc][:])
            rrow = cst_p.tile([1, 512], f32, tag=f"rr{qc}", name=f"rrow{qc}")
            nc.scalar.copy(rrow[:], pr[:])
            nc.sync.dma_start(rs_dram[0:1, qc * 512 : (qc + 1) * 512], rrow[:])
        rsum_t = cst_p.tile([P, 8], f32, tag="rst", name="rsum_t")
        nc.sync.dma_start(rsum_t[:, :], rs_dram[0, :].rearrange("(g p) -> p g", p=P))
        recip = cst_p.tile([P, 8], f32, tag="recip", name="recip")
        nc.vector.reciprocal(recip[:], rsum_t[:])

        for dc in range(KO8):
            pp = psA_p.tile([P, 512], f32, tag="psA", name="ppx")
            for ko in range(KO16):
                nc.tensor.matmul(
                    pp[:],
                    xn_sb[:, ko, dc * P : (dc + 1) * P],
                    e_sb[1][:, ko, :],
                    start=(ko == 0), stop=(ko == KO16 - 1),
                )
            nc.scalar.copy(px_sb[:, dc, 512:1024], pp[:])

        # ---- Phase 4: out[q, e] = PX^T.T Wv^T / rowsum + bv ---------------
        mul, add = mybir.AluOpType.mult, mybir.AluOpType.add

        def av_fuse(ps, g, c0, c1):
            # fused (psum * recip) + bv straight from PSUM on DVE, then DMA
            # (Pool/gpsimd cannot read PSUM on TRN2)
            o = out_p.tile([P, 512], bf16, tag="ost", name="ost")
            nc.vector.scalar_tensor_tensor(
                o[:, 0 : c1 - c0], ps[:], recip[:, g : g + 1],
                bvb_t[:, c0:c1], mul, add,
            )
            nc.sync.dma_start(out[g * P : (g + 1) * P, c0:c1], o[:, 0 : c1 - c0])

        for g in range(7):
            pb = psB_p.tile([P, 512], f32, tag="psB", name="avb")
            pc = psC_p.tile([P, 512], f32, tag="psC", name="avc")
            for dc in range(KO8):
                lh = px_sb[:, dc, g * P : (g + 1) * P]
                nc.tensor.matmul(
                    pb[:], lh, wv_sb[:, dc, 0:512],
                    start=(dc == 0), stop=(dc == KO8 - 1),
                )
                nc.tensor.matmul(
                    pc[:], lh, wv_sb[:, dc, 512:1024],
                    start=(dc == 0), stop=(dc == KO8 - 1),
                )
            if g < 6:
                # early groups have ~3.5us of slack: both halves fuse into
                # one tile and ship as a single 256KB dma (fewer dmas =
                # less teardown/queue work); only the tail groups split
                o = out_p.tile([P, D], bf16, tag="ost", name="ostm")
                mul, add = mybir.AluOpType.mult, mybir.AluOpType.add
                for half, ps in ((0, pb), (1, pc)):
                    nc.vector.scalar_tensor_tensor(
                        o[:, half * 512 : (half + 1) * 512],
                        ps[:], recip[:, g : g + 1],
                        bvb_t[:, half * 512 : (half + 1) * 512],
                        mul, add,
                    )
                nc.sync.dma_start(out[g * P : (g + 1) * P, :], o[:, :])
            else:
                av_fuse(pb, g, 0, 512)
                av_fuse(pc, g, 512, 1024)

        # last group de-interleaved: b completes (fuse+DMA overlap c's MMs),
        # then c in two 256-col accumulations so the post-last-MM tail is a
        # single small fuse + 64KB DMA.
        g = 7
        pb = psB_p.tile([P, 512], f32, tag="psB", name="avb")
        lhs = [px_sb[:, dc, g * P : (g + 1) * P] for dc in range(KO8)]
        for dc in range(KO8):
            nc.tensor.matmul(
                pb[:], lhs[dc], wv_sb[:, dc, 0:512],
                start=(dc == 0), stop=(dc == KO8 - 1),
            )
        av_fuse(pb, g, 0, 512)
        for half, c0 in ((0, 512), (1, 768)):
            pc = psC_p.tile([P, 256], f32, tag="psC", name=f"avc{half}")
            for dc in range(KO8):
                nc.tensor.matmul(
                    pc[:], lhs[dc], wv_sb[:, dc, c0 : c0 + 256],
                    start=(dc == 0), stop=(dc == KO8 - 1),
                )
            if half == 0:
                av_fuse(pc, g, c0, c0 + 256)
            else:
                # final piece: one fuse, then 32KB halves on BOTH rings —
                # the two ~600ns issues run concurrently on their queue
                # engines (scalar ring idle since phase 3) and the
                # transfers complete in parallel, shortening the tail
                o = out_p.tile([P, 256], bf16, tag="ost", name="ostz")
                nc.vector.scalar_tensor_tensor(
                    o[:, 0:256], pc[:], recip[:, g : g + 1],
                    bvb_t[:, c0 : c0 + 256], mul, add,
                )
                nc.sync.dma_start(
                    out[g * P : (g + 1) * P, c0 : c0 + 128], o[:, 0:128]
                )
                nc.scalar.dma_start(
                    out[g * P : (g + 1) * P, c0 + 128 : c0 + 256],
                    o[:, 128:256],
                )

    nc.finalize()
    return nc


def make_in_maps(x, Wq, bq, Wk, bk, Wv, bv):
    """Build the 8 per-core input maps from full inputs."""
    import ml_dtypes

    bf = ml_dtypes.bfloat16
    x = np.asarray(x, dtype=np.float32)
    inv8 = 1.0 / math.sqrt(D // 16)  # 1/sqrt(d_key=64) = 1/8
    # scores = x_q (Wq^T Wk) x_k^T / 8 + x_k.(Wk^T bq)/8 (+ softmax-invariant
    # per-query terms, dropped). Both folded into the query-side projection.
    M8 = (
        (np.asarray(Wq, np.float64).T @ np.asarray(Wk, np.float64)) * inv8
    ).astype(bf)
    w3 = (
        (np.asarray(Wk, np.float64).T @ np.asarray(bq, np.float64)) * inv8
    ).astype(np.float32)
    w3_np = np.ascontiguousarray(w3.reshape(KO8, P).T)
    wvT = np.ascontiguousarray(np.asarray(Wv, np.float32).T.astype(bf))
    bvb = np.ascontiguousarray(
        np.broadcast_to(np.asarray(bv, np.float32), (P, D))
    )
    in_maps = []
    for c in range(N_CORES):
        b, h = c // 2, c % 2
        # rotate the key axis by h*SQ so this core's queries are always
        # columns 0:SQ of xT; attention is permutation-invariant over keys
        # as long as xT (scores lhsT) and xn (PX lhsT) rotate together.
        xb = np.roll(x[b], -h * SQ, axis=0)
        in_maps.append(
            {
                "xT": np.ascontiguousarray(xb.T.astype(bf)),
                "xn": np.ascontiguousarray(xb.astype(bf)),
                "mT": M8,
                "wvT": wvT,
                "w3": w3_np,
                "bvb": bvb,
            }
        )
    return in_maps


_NC_CACHE = None


def get_nc():
    global _NC_CACHE
    if _NC_CACHE is None:
        _NC_CACHE = build_bass()
    return _NC_CACHE


def kernel(x, Wq, bq, Wk, bk, Wv, bv, **run_kwargs):
    from concourse.bass_utils import run_bass_kernel_spmd

    nc = get_nc()
    in_maps = make_in_maps(x, Wq, bq, Wk, bk, Wv, bv)
    res = run_bass_kernel_spmd(
        nc, in_maps, core_ids=list(range(N_CORES)), **run_kwargs
    )
    out = np.empty((B, S, D), dtype=np.float32)
    for c in range(N_CORES):
        b, h = c // 2, c % 2
        out[b, h * SQ : (h + 1) * SQ, :] = np.asarray(
            res.results[c]["out"], dtype=np.float32
        )
    if run_kwargs.get("trace"):
        kernel.last_results = res
    return out



# revision 33
# speedup vs baseline: 1.0199x; 1.0199x over previous
"""Full-width attention (B=4, S=2048, D=1024, no head split) on 8 TRN2 cores.

Sharding: data-parallel over (batch, query-half) -> 8 shards. Core c handles
batch b = c//2, query rows [h*1024, (h+1)*1024) with h = c%2.

Algorithm (FLOP-minimal per core: 12.9 GF = 768 N=512 matmuls, bf16):
  - Fold Wq/Wk into the QUERY side: Q'' = (x_q M + w3) / 8 with M = Wq^T Wk
    and w3 = Wk^T bq. scores^T[k,q] = sum_e x[k,e] Q''[q,e] needs no key
    projection. Softmax without max-subtraction (|scores| <= ~25).
  - out[q,e] = (x^T E)^T Wv^T scaled by 1/rowsum + bv.
  - All big matmuls bf16 x bf16 (216 ns/MM at N=512, warm): PE streams
    gap-free at ~99% occupancy for the whole compute window.

Round-2/3/4 changes vs the 187.0us version (trace-driven; now ~182.7-183.4
in the 2.4GHz chip state):
  - Warmup = 9 MMs, tuned so the ones-warmup ends exactly at the first
    m-chunk arrival (~11.9us, HBM-contention-pinned across the 8 cores)
    AND pre-burns the full ~3.4us HAM window, so ko0 runs warm at 216
    ns/MM instead of cold at 427. After that the 768-MM stream is
    gap-free to the end (verified <0.5us total stalls).
  - xt loads split into qc-halves and the xt-b half gated behind an
    m7 blocker read: the scalar ring's critical piece is 1KB/ko (vs m's
    2KB) and the ring goes idle after xt-a, so the SDMA round-robin
    gives the m ring the full HBM share for m2-m7 (~1us/ko faster
    arrival; lastMM moved ~1us earlier, reproducibly). A finer JIT
    throttle (blocker per xt-a chunk) over-serialized the scalar queue
    and delayed the qc0 evacuations — net loss, don't revisit.
  - The xt-tail/wv/xn DMA issues moved off the Scalar queue: each issue
    costs 600-800ns of queue-engine time and was serializing with the
    qc0 PSUM evacuations (ACTIVATE), stalling qc1's first MMs ~1us.
  - qc0/qc1 evacuations alternate Scalar (activation+bias) and DVE
    (tensor_scalar_add): evac rate 432 -> ~216 ns/bank matches the MM
    consume rate at the sweep boundary.
  - Phase 4 last group de-interleaved (b fully, then c split in 2x256)
    so the post-last-MM tail is one small fuse + 64KB DMA instead of
    two serialized 512-col fuses + 128KB DMAs.
  - Round 4: xt-b as 2 coarse dmas and groups g0-g5's output as one
    256KB dma each (both have multi-us slack): fewer dmas reduce queue
    and teardown bookkeeping; measured best dropped 183.3 -> 182.7.
    Re-tried m0 half-split WITH the blocker (start ~11.8 vs 11.9): the
    extra dma's completion overhead stalls ko2-3 ~1.4us on bad runs —
    net negative (183.6 mean vs 183.1), reverted.
  - Round 5: the final 64KB out piece ships as two 32KB dmas on BOTH
    rings (sync + scalar issues run concurrently; scalar ring idle from
    phase 3 on) — parallelizes the ~600ns issue and the transfer in the
    exposed tail. Config locked at 182.7-183.4us.
Measured invariants (don't re-litigate): per-NC HBM ~320-360GB/s total,
~160/ring steady, first-chunk completion ~11.9us (ring-arm + contention);
a 3rd DMA path (gpsimd SWDGE) causes 3-way contention, stalls ko1-3 by
~5us and re-throttles HAM -- strictly worse. Phase-1's end is pinned by
TOTAL 4MB arrival (~25.8us), so no phase-1 restructure (finer chunks,
eo/qc generations) can beat the current ko-outer sweep. fp8/DoubleRow is
precision-infeasible everywhere (needs <=2e-2 l2; e4m3 alone adds ~3.6%).
Periodic +163ns PE hiccups every 10.79us (~2.3us total) are firmware.
Run-to-run: chip sometimes drops to 2.0GHz (P0 power state) -> all MMs
259ns and ~+35us total; not code-dependent, ignore those samples.
"""

import math
from contextlib import ExitStack

import numpy as np

P = 128
B, S, D = 4, 2048, 1024
SQ = 1024  # query rows per core
KO8 = 8  # 1024 contraction / 128
KO16 = 16  # 2048 contraction / 128
N_CORES = 8
N_WARM = 8


def build_bass():
    from concourse import bacc
    import concourse.mybir as mybir
    from concourse.tile import TileContext

    f32 = mybir.dt.float32
    f32r = mybir.dt.float32r
    bf16 = mybir.dt.bfloat16
    AF = mybir.ActivationFunctionType

    nc = bacc.Bacc(
        "TRN2",
        target_bir_lowering=False,
        debug=False,
        enable_asserts=False,
        num_devices=N_CORES,
    )

    xT = nc.dram_tensor("xT", [D, S], bf16, kind="ExternalInput")
    xn = nc.dram_tensor("xn", [S, D], bf16, kind="ExternalInput")
    mT = nc.dram_tensor("mT", [D, D], bf16, kind="ExternalInput")
    wvT = nc.dram_tensor("wvT", [D, D], bf16, kind="ExternalInput")
    w3 = nc.dram_tensor("w3", [P, KO8], f32, kind="ExternalInput")
    bvb = nc.dram_tensor("bvb", [P, D], f32, kind="ExternalInput")
    # bf16 output halves the out-DMA traffic (the tail's exposed transfer);
    # host converts back to f32 — quantization ~0.2%, well inside margin
    out = nc.dram_tensor("out", [SQ, D], bf16, kind="ExternalOutput")

    xT_r = xT[:, :].rearrange("(ko p) s -> p ko s", p=P)
    xn_r = xn[:, :].rearrange("(ko p) d -> p ko d", p=P)
    mT_r = mT[:, :].rearrange("(ko p) e -> p ko e", p=P)
    wvT_r = wvT[:, :].rearrange("(ko p) e -> p ko e", p=P)

    with TileContext(nc) as tc, ExitStack() as ctx:
        cst_p = ctx.enter_context(tc.tile_pool(name="cst", bufs=1))
        big_p = ctx.enter_context(tc.tile_pool(name="big", bufs=1))
        out_p = ctx.enter_context(tc.tile_pool(name="osp", bufs=3))
        psA_p = ctx.enter_context(tc.tile_pool(name="psA", bufs=3, space="PSUM"))
        psB_p = ctx.enter_context(tc.tile_pool(name="psB", bufs=2, space="PSUM"))
        psC_p = ctx.enter_context(tc.tile_pool(name="psC", bufs=2, space="PSUM"))
        psR_p = ctx.enter_context(tc.tile_pool(name="psR", bufs=1, space="PSUM"))
        dram_p = ctx.enter_context(tc.tile_pool(name="drp", bufs=1, space="DRAM"))

        # warmup operand comes from a memset, not a DMA (walrus rejects
        # memset on f32r tiles, so memset f32 and bitcast for the PE)
        ones_f = cst_p.tile([P, 512], f32, tag="ones", name="ones_f")
        nc.vector.memset(ones_f[:], 1.0)
        ones_t = ones_f[:, :].bitcast(f32r)
        w3_t = cst_p.tile([P, KO8], f32, tag="w3", name="w3_t")
        bvb_t = cst_p.tile([P, D], f32, tag="bvb", name="bvb_t")

        # big residents
        xt_sb = big_p.tile([P, KO8, S], bf16, tag="xt", name="xt_sb")
        xn_sb = big_p.tile([P, KO16, D], bf16, tag="xn", name="xn_sb")
        m_sb = big_p.tile([P, KO8, D], bf16, tag="m", name="m_sb")
        wv_sb = big_p.tile([P, KO8, D], bf16, tag="wv", name="wv_sb")
        qt_sb = big_p.tile([P, KO8, SQ], bf16, tag="qt", name="qt_sb")
        e_sb = [
            big_p.tile([P, KO16, 512], bf16, tag=f"E{qc}", name=f"e_sb{qc}")
            for qc in range(2)
        ]
        px_sb = big_p.tile([P, KO8, SQ], bf16, tag="px", name="px_sb")
        racc = [
            cst_p.tile([P, 512], f32r, tag=f"racc{qc}", name=f"racc{qc}")
            for qc in range(2)
        ]
        rs_dram = dram_p.tile([1, SQ], f32, tag="rsd", name="rs_dram")

        # Phase-1 feed: per-ko full-width chunks (one dma each — finer splits
        # pay ~0.4us/dma completion overhead), m on the sync ring and x^T
        # query columns on the scalar ring in parallel. All non-phase-1
        # loads ride the SYNC ring behind the m chunks: their ~700ns issue
        # slots must not serialize with the qc0 evacuations on the Scalar
        # engine queue, and their transfers stay behind the critical chunks
        # in ring-FIFO order. (SWDGE as a 3rd path is a net loss — per-NC
        # HBM caps ~320-360 GB/s total.)
        # xt comes as qc-halves: the qc0 sweep needs only cols 0:512 per ko,
        # so the scalar ring's per-ko critical piece is 1KB/partition vs the
        # m ring's 2KB — the m ring alone gates the sweep, removing the
        # max-of-two-rings arrival jitter. The xt-b tail (+8 dmas of
        # overhead) is only needed by the qc1 sweep, which has ~12us slack.
        # m0 halves ride DIFFERENT rings: ring1's first dma shrinks to
        # 128KB (so m1.. arrive strictly earlier than with a full m0) and
        # m0b slots second on the scalar ring ahead of the slack-rich
        # xt-a tail — the first-MM gate drops ~11.9 -> ~11.5us.
        nc.sync.dma_start(m_sb[:, 0, 0:512], mT_r[:, 0, 0:512])
        nc.scalar.dma_start(xt_sb[:, 0, 0:512], xT_r[:, 0, 0:512])
        nc.scalar.dma_start(m_sb[:, 0, 512:D], mT_r[:, 0, 512:D])
        for ko in range(1, KO8):
            nc.sync.dma_start(m_sb[:, ko, :], mT_r[:, ko, :])
            nc.scalar.dma_start(xt_sb[:, ko, 0:512], xT_r[:, ko, 0:512])
        # small consts on the scalar ring (w3 first used ~22us in)
        nc.scalar.dma_start(w3_t[:], w3[:, :])
        # blocker: hold the xt-b issues until the last m chunk has landed.
        # SDMA engines round-robin only among queues WITH work, so an idle
        # scalar ring gives the m ring the full HBM share during the qc0
        # sweep's critical window (m paces 1.6-1.7us/ko vs 1.73 consumed).
        # xt-b is first needed by the qc1 sweep (~12us of slack).
        blk_t = cst_p.tile([1, 1], f32, tag="blk", name="blk_t")
        nc.scalar.copy(blk_t[:], m_sb[0:1, KO8 - 1, D - 1 : D])
        # xt-b as 2 coarse dmas: it has >5us of slack before the qc1 sweep
        # consumes it, and fewer dmas mean less issue time ahead of the
        # qc0 evacuations on this queue (and a hair less teardown work)
        for kp in range(2):
            nc.scalar.dma_start(
                xt_sb[:, 4 * kp : 4 * kp + 4, 512:SQ],
                xT_r[:, 4 * kp : 4 * kp + 4, 512:SQ],
            )
        nc.scalar.dma_start(bvb_t[:], bvb[:, :])
        # non-critical bulk loads: sync ring, behind the m halves
        for kp in range(2):
            nc.sync.dma_start(
                xt_sb[:, 4 * kp : 4 * kp + 4, SQ:S],
                xT_r[:, 4 * kp : 4 * kp + 4, SQ:S],
            )
        nc.sync.dma_start(wv_sb[:, :, :], wvT_r[:, :, :])
        for kp in range(2):
            nc.sync.dma_start(
                xn_sb[:, 8 * kp : 8 * kp + 8, :], xn_r[:, 8 * kp : 8 * kp + 8, :]
            )

        # PE warm-up on the ones tile: 9 MMs end right at the first m-chunk
        # arrival (~11.9us) and pre-burn the full ~3.4us HAM window with
        # ~0.6us margin, so the real stream starts warm (216ns/MM).
        warm = psR_p.tile([1, 512], f32, tag="psR", name="warm")
        for _ in range(N_WARM):
            nc.tensor.matmul(warm[:], ones_t[:, 0:1], ones_t[:, :])

        # ---- Phase 1: Q''T[e, q] = M^T x_q^T + w3 (scaled by 1/8 on host) --
        # ko-OUTER with all 8 eo accumulations held open across the full
        # PSUM bank set: each ko step needs only chunk ko of m/x^T, so the
        # whole sweep paces with the DMA chunk arrivals. (4-pass eo/qc-half
        # restructures with split/merged DMAs were measured at 184.9-187.9
        # vs 184.0-186.3 for this layout: per-dma completion overhead
        # (~0.4us) and cross-ring HBM contention eat the theoretical gain.)
        def q_sweep(qc):
            banks = [
                psA_p.tile([P, 512], f32, tag="psA", name=f"qp{qc}a{i}")
                for i in range(3)
            ] + [
                psB_p.tile([P, 512], f32, tag="psB", name=f"qp{qc}b{i}")
                for i in range(2)
            ] + [
                psC_p.tile([P, 512], f32, tag="psC", name=f"qp{qc}c{i}")
                for i in range(2)
            ] + [psR_p.tile([P, 512], f32, tag="psR", name=f"qp{qc}r")]
            for ko in range(KO8):
                for eo in range(KO8):
                    nc.tensor.matmul(
                        banks[eo][:],
                        m_sb[:, ko, eo * P : (eo + 1) * P],
                        xt_sb[:, ko, qc * 512 : (qc + 1) * 512],
                        start=(ko == 0), stop=(ko == KO8 - 1),
                    )
            # evacuations alternate Scalar/DVE so the next sweep's first MMs
            # (which reuse these banks) aren't rate-limited by one engine
            for eo in range(KO8):
                dst = qt_sb[:, eo, qc * 512 : (qc + 1) * 512]
                if eo % 2 == 0:
                    nc.scalar.activation(
                        dst, banks[eo][:], AF.Identity, bias=w3_t[:, eo : eo + 1]
                    )
                else:
                    nc.vector.tensor_scalar_add(
                        dst, banks[eo][:], w3_t[:, eo : eo + 1]
                    )

        q_sweep(0)
        q_sweep(1)

        # ---- Phase 2: scores^T -> exp -> E (bf16), rowsum acc on DVE ------
        for kidx in range(KO16):
            pa = psA_p.tile([P, 512], f32, tag="psA", name="spa")
            pb = psB_p.tile([P, 512], f32, tag="psB", name="spb")
            for eo in range(KO8):
                lh = xt_sb[:, eo, kidx * P : (kidx + 1) * P]
                nc.tensor.matmul(
                    pa[:], lh, qt_sb[:, eo, 0:512],
                    start=(eo == 0), stop=(eo == KO8 - 1),
                )
                nc.tensor.matmul(
                    pb[:], lh, qt_sb[:, eo, 512:1024],
                    start=(eo == 0), stop=(eo == KO8 - 1),
                )
            nc.scalar.activation(e_sb[0][:, kidx, :], pa[:], AF.Exp)
            nc.scalar.activation(e_sb[1][:, kidx, :], pb[:], AF.Exp)
            for qc in range(2):
                if kidx == 0:
                    nc.vector.tensor_copy(racc[qc][:], e_sb[qc][:, 0, :])
                else:
                    nc.vector.tensor_add(
                        racc[qc][:], racc[qc][:], e_sb[qc][:, kidx, :]
                    )

        # ---- Phase 3: PX^T[d, q] = sum_k x[k, d] E[k, q] -------------------
        for dc in range(KO8):
            pp = psA_p.tile([P, 512], f32, tag="psA", name="ppx")
            for ko in range(KO16):
                nc.tensor.matmul(
                    pp[:],
                    xn_sb[:, ko, dc * P : (dc + 1) * P],
                    e_sb[0][:, ko, :],
                    start=(ko == 0), stop=(ko == KO16 - 1),
                )
            nc.scalar.copy(px_sb[:, dc, 0:512], pp[:])

        # rowsum partition-reduce + [1,1024] -> [128,8] recip via DRAM bounce
        # (PE cost ~2 tiny matmuls; bounce hides under PX)
        for qc in range(2):
            pr = psR_p.tile([1, 512], f32, tag="psR", name="pr")
            nc.tensor.matmul(pr[:], ones_t[:, 0:1], racc[qc][:])
            rrow = cst_p.tile([1, 512], f32, tag=f"rr{qc}", name=f"rrow{qc}")
            nc.scalar.copy(rrow[:], pr[:])
            nc.sync.dma_start(rs_dram[0:1, qc * 512 : (qc + 1) * 512], rrow[:])
        rsum_t = cst_p.tile([P, 8], f32, tag="rst", name="rsum_t")
        nc.sync.dma_start(rsum_t[:, :], rs_dram[0, :].rearrange("(g p) -> p g", p=P))
        recip = cst_p.tile([P, 8], f32, tag="recip", name="recip")
        nc.vector.reciprocal(recip[:], rsum_t[:])

        for dc in range(KO8):
            pp = psA_p.tile([P, 512], f32, tag="psA", name="ppx")
            for ko in range(KO16):
                nc.tensor.matmul(
                    pp[:],
                    xn_sb[:, ko, dc * P : (dc + 1) * P],
                    e_sb[1][:, ko, :],
                    start=(ko == 0), stop=(ko == KO16 - 1),
                )
            nc.scalar.copy(px_sb[:, dc, 512:1024], pp[:])

        # ---- Phase 4: out[q, e] = PX^T.T Wv^T / rowsum + bv ---------------
        mul, add = mybir.AluOpType.mult, mybir.AluOpType.add

        def av_fuse(ps, g, c0, c1):
            # fused (psum * recip) + bv straight from PSUM on DVE, then DMA
            # (Pool/gpsimd cannot read PSUM on TRN2)
            o = out_p.tile([P, 512], bf16, tag="ost", name="ost")
            nc.vector.scalar_tensor_tensor(
                o[:, 0 : c1 - c0], ps[:], recip[:, g : g + 1],
                bvb_t[:, c0:c1], mul, add,
            )
            nc.sync.dma_start(out[g * P : (g + 1) * P, c0:c1], o[:, 0 : c1 - c0])

        for g in range(7):
            pb = psB_p.tile([P, 512], f32, tag="psB", name="avb")
            pc = psC_p.tile([P, 512], f32, tag="psC", name="avc")
            for dc in range(KO8):
                lh = px_sb[:, dc, g * P : (g + 1) * P]
                nc.tensor.matmul(
                    pb[:], lh, wv_sb[:, dc, 0:512],
                    start=(dc == 0), stop=(dc == KO8 - 1),
                )
                nc.tensor.matmul(
                    pc[:], lh, wv_sb[:, dc, 512:1024],
                    start=(dc == 0), stop=(dc == KO8 - 1),
                )
            if g < 6:
                # early groups have ~3.5us of slack: both halves fuse into
                # one tile and ship as a single 256KB dma (fewer dmas =
                # less teardown/queue work); only the tail groups split
                o = out_p.tile([P, D], bf16, tag="ost", name="ostm")
                mul, add = mybir.AluOpType.mult, mybir.AluOpType.add
                for half, ps in ((0, pb), (1, pc)):
                    nc.vector.scalar_tensor_tensor(
                        o[:, half * 512 : (half + 1) * 512],
                        ps[:], recip[:, g : g + 1],
                        bvb_t[:, half * 512 : (half + 1) * 512],
                        mul, add,
                    )
                nc.sync.dma_start(out[g * P : (g + 1) * P, :], o[:, :])
            else:
                av_fuse(pb, g, 0, 512)
                av_fuse(pc, g, 512, 1024)

        # last group de-interleaved: b completes (fuse+DMA overlap c's MMs),
        # then c in two 256-col accumulations so the post-last-MM tail is a
        # single small fuse + 64KB DMA.
        g = 7
        pb = psB_p.tile([P, 512], f32, tag="psB", name="avb")
        lhs = [px_sb[:, dc, g * P : (g + 1) * P] for dc in range(KO8)]
        for dc in range(KO8):
            nc.tensor.matmul(
                pb[:], lhs[dc], wv_sb[:, dc, 0:512],
                start=(dc == 0), stop=(dc == KO8 - 1),
            )
        av_fuse(pb, g, 0, 512)
        for half, c0 in ((0, 512), (1, 768)):
            pc = psC_p.tile([P, 256], f32, tag="psC", name=f"avc{half}")
            for dc in range(KO8):
                nc.tensor.matmul(
                    pc[:], lhs[dc], wv_sb[:, dc, c0 : c0 + 256],
                    start=(dc == 0), stop=(dc == KO8 - 1),
                )
            if half == 0:
                av_fuse(pc, g, c0, c0 + 256)
            else:
                # final piece: one fuse, then 32KB halves on BOTH rings —
                # the two ~600ns issues run concurrently on their queue
                # engines (scalar ring idle since phase 3) and the
                # transfers complete in parallel, shortening the tail
                o = out_p.tile([P, 256], bf16, tag="ost", name="ostz")
                nc.vector.scalar_tensor_tensor(
                    o[:, 0:256], pc[:], recip[:, g : g + 1],
                    bvb_t[:, c0 : c0 + 256], mul, add,
                )
                nc.sync.dma_start(
                    out[g * P : (g + 1) * P, c0 : c0 + 128], o[:, 0:128]
                )
                nc.scalar.dma_start(
                    out[g * P : (g + 1) * P, c0 + 128 : c0 + 256],
                    o[:, 128:256],
                )

    nc.finalize()
    return nc


def make_in_maps(x, Wq, bq, Wk, bk, Wv, bv):
    """Build the 8 per-core input maps from full inputs."""
    import ml_dtypes

    bf = ml_dtypes.bfloat16
    x = np.asarray(x, dtype=np.float32)
    inv8 = 1.0 / math.sqrt(D // 16)  # 1/sqrt(d_key=64) = 1/8
    # scores = x_q (Wq^T Wk) x_k^T / 8 + x_k.(Wk^T bq)/8 (+ softmax-invariant
    # per-query terms, dropped). Both folded into the query-side projection.
    M8 = (
        (np.asarray(Wq, np.float64).T @ np.asarray(Wk, np.float64)) * inv8
    ).astype(bf)
    w3 = (
        (np.asarray(Wk, np.float64).T @ np.asarray(bq, np.float64)) * inv8
    ).astype(np.float32)
    w3_np = np.ascontiguousarray(w3.reshape(KO8, P).T)
    wvT = np.ascontiguousarray(np.asarray(Wv, np.float32).T.astype(bf))
    bvb = np.ascontiguousarray(
        np.broadcast_to(np.asarray(bv, np.float32), (P, D))
    )
    in_maps = []
    for c in range(N_CORES):
        b, h = c // 2, c % 2
        # rotate the key axis by h*SQ so this core's queries are always
        # columns 0:SQ of xT; attention is permutation-invariant over keys
        # as long as xT (scores lhsT) and xn (PX lhsT) rotate together.
        xb = np.roll(x[b], -h * SQ, axis=0)
        in_maps.append(
            {
                "xT": np.ascontiguousarray(xb.T.astype(bf)),
                "xn": np.ascontiguousarray(xb.astype(bf)),
                "mT": M8,
                "wvT": wvT,
                "w3": w3_np,
                "bvb": bvb,
            }
        )
    return in_maps


_NC_CACHE = None


def get_nc():
    global _NC_CACHE
    if _NC_CACHE is None:
        _NC_CACHE = build_bass()
    return _NC_CACHE


def kernel(x, Wq, bq, Wk, bk, Wv, bv, **run_kwargs):
    from concourse.bass_utils import run_bass_kernel_spmd

    nc = get_nc()
    in_maps = make_in_maps(x, Wq, bq, Wk, bk, Wv, bv)
    res = run_bass_kernel_spmd(
        nc, in_maps, core_ids=list(range(N_CORES)), **run_kwargs
    )
    out = np.empty((B, S, D), dtype=np.float32)
    for c in range(N_CORES):
        b, h = c // 2, c % 2
        out[b, h * SQ : (h + 1) * SQ, :] = np.asarray(
            res.results[c]["out"], dtype=np.float32
        )
    if run_kwargs.get("trace"):
        kernel.last_results = res
    return out

